# revision 1
# baseline (speedup 1.0000x reference)
"""Trainium2 Bass kernel for nn_NodeProcessor (GNN message passing).

Computation (per reference):
    agg = segment_sum(edge_attr, col=edge_index[1], N)      # [N, 64]
    h = relu(concat([x, agg]) @ W0 + b0)
    h = relu(h @ W1 + b1)
    h = h @ W2 + b2
    out = layernorm(h) * ln_g + ln_b + x

Distribution: destination-sharded edges (no collectives). Core c owns nodes
[c*12544, (c+1)*12544); the host routes each edge to the core owning its
destination (that routing IS the sharding step), so every core computes its
full [12544, 64] aggregate locally, runs the node MLP on its shard, and the
host concatenates the 8 outputs.

Per-core device kernel (identical SPMD program):
  Scatter: edges grouped by 64-node window (196/core), padded to a uniform
  K 128-edge tiles per window. Edge features ship as stacked bf16 hi|lo
  (error ~2^-17). Per tile, ONE matmul with the 128x128 hi|lo tile as the
  stationary operand and a [128, 64] one-hot (edge -> window-node) as the
  moving operand accumulates [hi_feats|lo_feats, 64 nodes] into fp32 PSUM.
  One-hots for a whole window are built by a single DVE is_equal whose
  operands are all packed 2-byte strides (iota replicated K times) to hit
  the DVE 2x perf mode. The hi/lo halves are merged for free inside the
  layer-0 matmul: h0 += W0a^T@agg_hi + W0a^T@agg_lo via a duplicated-W0a
  weight tile on partitions 64:127.
  MLP+LN: software-pipelined stages (layer0 / layer1 / layer2 / transpose /
  ln-stats / residual+store) skewed across the scatter loop so no engine's
  in-order queue ever stalls on a same-tile dependency.
"""

import numpy as np
import ml_dtypes
from contextlib import ExitStack

from concourse import bacc, mybir
from concourse.tile import TileContext
from concourse.bass_utils import run_bass_kernel_spmd

BF16 = ml_dtypes.bfloat16

N_NODES = 100000
N_EDGES = 1200000
D = 64          # d_node == d_edge
H = 128         # hidden
EPS = 1e-5
NCORES = 8
P = 128
WIN = 64        # nodes per scatter window
NW = 196        # windows per core
MT = 98         # MLP tiles (128 nodes each) per core
NPC = NW * WIN  # 12544 nodes per core
QW = 4          # windows per attr DMA
OB = 4          # MLP tiles per output DMA

_prog_cache: dict[int, object] = {}
last_results = None


def _build_program(K: int, ablate: str = "", reps: int = 1):
    nc = bacc.Bacc("TRN2", target_bir_lowering=False, debug=False,
                   num_devices=NCORES)
    f32 = mybir.dt.float32
    bf16 = mybir.dt.bfloat16
    AF = mybir.ActivationFunctionType
    ALU = mybir.AluOpType
    ab = set(ablate.split(",")) if ablate else set()

    T = NW * K  # edge tiles per core

    attr_d = nc.dram_tensor("attr", [P, T * P], bf16, kind="ExternalInput").ap()
    r_d = nc.dram_tensor("r", [P, T], bf16, kind="ExternalInput").ap()
    iot_d = nc.dram_tensor("iota_rep", [P, WIN * K], bf16, kind="ExternalInput").ap()
    cst_d = nc.dram_tensor("cst", [P, 654], f32, kind="ExternalInput").ap()
    f32r = mybir.dt.float32r
    wr_d = nc.dram_tensor("wr", [P, 576], f32, kind="ExternalInput").ap()
    xT_d = nc.dram_tensor("xT", [D, NPC], f32, kind="ExternalInput").ap()
    xsw_d = nc.dram_tensor("x_sw", [P, MT * D], f32, kind="ExternalInput").ap()
    out_d = nc.dram_tensor("out_sw", [P, MT * D], f32, kind="ExternalOutput").ap()

    with TileContext(nc) as tc, ExitStack() as ctx:
        const = ctx.enter_context(tc.tile_pool(name="const", bufs=1))
        sb = ctx.enter_context(tc.tile_pool(name="sb", bufs=3))
        winp = ctx.enter_context(tc.tile_pool(name="winp", bufs=4))
        ohp = ctx.enter_context(tc.tile_pool(name="ohp", bufs=8))
        aggp = ctx.enter_context(tc.tile_pool(name="aggp", bufs=4))
        outp = ctx.enter_context(tc.tile_pool(name="outp", bufs=3))
        ps_win = ctx.enter_context(tc.tile_pool(name="ps_win", bufs=2, space="PSUM"))
        ps_h0 = ctx.enter_context(tc.tile_pool(name="ps_h0", bufs=2, space="PSUM"))
        ps_h1 = ctx.enter_context(tc.tile_pool(name="ps_h1", bufs=1, space="PSUM"))
        ps_h2 = ctx.enter_context(tc.tile_pool(name="ps_h2", bufs=1, space="PSUM"))
        ps_nm = ctx.enter_context(tc.tile_pool(name="ps_nm", bufs=2, space="PSUM"))

        # ---- constants (one packed f32 DMA + bf16 iota/r + x tensors) ----
        cst = const.tile([P, 654], f32, tag="cst")
        nc.sync.dma_start(out=cst[:], in_=cst_d[:])
        i64_t = cst[0:D, 448:512]
        g_t = cst[:, 512:576]
        bln_t = cst[:, 576:640]
        b0_t = cst[:, 640:641]
        b1_t = cst[:, 641:642]
        b2_t = cst[0:D, 642:643]
        eps_t = cst[:, 643:644]
        wr = const.tile([P, 576], f32, tag="wr")
        nc.sync.dma_start(out=wr[:], in_=wr_d[:])
        w0x_r = wr[0:D, 0:H]
        w0ahi_r = wr[:, 128:256]   # rows 0:64 = W0a, rows 64:128 = 0
        w0alo_r = wr[:, 256:384]   # rows 0:64 = 0, rows 64:128 = W0a
        w1_r = wr[:, 384:512]
        w2_r = wr[:, 512:576]

        iota_t = const.tile([P, WIN * K], bf16, tag="iota")
        nc.sync.dma_start(out=iota_t[:], in_=iot_d[:])
        r_all = const.tile([P, T], bf16, tag="r_all")
        nc.sync.dma_start(out=r_all[:], in_=r_d[:])
        xT_s = const.tile([D, NPC], f32, tag="xT")
        nc.sync.dma_start(out=xT_s[:], in_=xT_d[:])
        xsw_s = const.tile([P, MT * D], f32, tag="xsw")
        nc.sync.dma_start(out=xsw_s[:], in_=xsw_d[:])

        # aggF tiles: [128 (hi|lo feats), 128 nodes] per MLP tile
        agg_tiles = {}
        out_tiles = {}

        # ---- scatter: one window ----
        def scatter_window(w, win, woff):
            """win: quad attr tile; woff: window index within the quad."""
            oh = ohp.tile([P, WIN * K], bf16, tag="oh")
            r_b = r_all[:, w * K:(w + 1) * K].rearrange(
                "p (o k) -> p o k", o=1).to_broadcast([P, WIN, K])
            if "nocompare" not in ab:
                nc.vector.tensor_tensor(
                    out=oh[:].rearrange("p (n k) -> p n k", k=K),
                    in0=r_b,
                    in1=iota_t[:].rearrange("p (n k) -> p n k", k=K),
                    op=ALU.is_equal)
            acc = ps_win.tile([P, WIN], f32, tag="acc")
            oh_v = oh[:].rearrange("p (n k) -> p k n", k=K)
            for t in range(K) if "nomm" not in ab else []:
                nc.tensor.matmul(
                    out=acc[:],
                    lhsT=win[:, (woff * K + t) * P:(woff * K + t + 1) * P],
                    rhs=oh_v[:, t, :],
                    start=(t == 0), stop=(t == K - 1))
            mb, quarter = divmod(w, QW)
            if mb not in agg_tiles:
                agg_tiles[mb] = aggp.tile([P, QW * WIN], f32, tag="aggF",
                                          name=f"aggF{mb}")
            nc.scalar.activation(
                out=agg_tiles[mb][:, quarter * WIN:(quarter + 1) * WIN],
                in_=acc[:], func=AF.Copy)

        # ---- MLP stages over 256-node mega-tiles (one per quad) ----
        MB = P * 2  # 256 nodes per mega-tile

        def st2(mb, _):
            aggF = agg_tiles.pop(mb)
            h0 = ps_h0.tile([H, MB], f32, tag="h0")
            nc.tensor.matmul(out=h0[:], lhsT=w0x_r,
                             rhs=xT_s[:, mb * MB:(mb + 1) * MB],
                             start=True, stop=False)
            nc.tensor.matmul(out=h0[:], lhsT=w0ahi_r, rhs=aggF[:],
                             start=False, stop=False)
            nc.tensor.matmul(out=h0[:], lhsT=w0alo_r, rhs=aggF[:],
                             start=False, stop=True)
            h0s = sb.tile([H, MB], f32, tag="h0s", name=f"h0s{mb}")
            nc.scalar.activation(out=h0s[:], in_=h0[:], func=AF.Relu, bias=b0_t)
            return h0s

        def st3(mb, h0s):
            h1 = ps_h1.tile([H, MB], f32, tag="h1")
            nc.tensor.matmul(out=h1[:], lhsT=w1_r, rhs=h0s[:],
                             start=True, stop=True)
            h1s = sb.tile([H, MB], f32, tag="h1s", name=f"h1s{mb}")
            nc.scalar.activation(out=h1s[:], in_=h1[:], func=AF.Relu, bias=b1_t)
            return h1s

        def st4(mb, h1s):
            h2 = ps_h2.tile([D, MB], f32, tag="h2")
            nc.tensor.matmul(out=h2[:], lhsT=w2_r, rhs=h1s[:],
                             start=True, stop=True)
            h2s = sb.tile([D, MB], f32, tag="h2s", name=f"h2s{mb}")
            nc.scalar.activation(out=h2s[:], in_=h2[:], func=AF.Identity, bias=b2_t)
            nm = ps_nm.tile([P, P], f32, tag="nm", name=f"nm{mb}")
            nc.tensor.transpose(out=nm[:, 0:D], in_=h2s[:, 0:P], identity=i64_t)
            nc.tensor.transpose(out=nm[:, D:P], in_=h2s[:, P:MB], identity=i64_t)
            return nm

        def st5(mb, nm):
            ys = []
            for h in range(2):
                nmh = nm[:, h * D:(h + 1) * D]
                st6t = sb.tile([P, 6], f32, tag="st6")
                nc.vector.bn_stats(out=st6t[:], in_=nmh)
                mv = sb.tile([P, 2], f32, tag="mv")
                nc.vector.bn_aggr(out=mv[:], in_=st6t[:])
                std = sb.tile([P, 1], f32, tag="std")
                nc.scalar.activation(out=std[:], in_=mv[:, 1:2], func=AF.Sqrt,
                                     bias=eps_t)
                rstd = sb.tile([P, 1], f32, tag="rstd")
                nc.vector.reciprocal(out=rstd[:], in_=std[:])
                y0 = sb.tile([P, D], f32, tag="y0", name=f"y0{mb}_{h}")
                nc.vector.tensor_scalar(out=y0[:], in0=nmh, scalar1=mv[:, 0:1],
                                        scalar2=rstd[:], op0=ALU.subtract,
                                        op1=ALU.mult)
                ys.append(y0)
            return ys

        def st6(mb, ys):
            ob = outp.tile([P, MB // P * D], f32, tag="outb", name=f"outb{mb}")
            for h in range(2):
                m = 2 * mb + h
                y1 = sb.tile([P, D], f32, tag="y1")
                nc.gpsimd.tensor_tensor(out=y1[:], in0=ys[h][:], in1=g_t,
                                        op=ALU.mult)
                y2 = sb.tile([P, D], f32, tag="y2")
                nc.gpsimd.tensor_tensor(out=y2[:], in0=y1[:], in1=bln_t, op=ALU.add)
                nc.gpsimd.tensor_tensor(out=ob[:, h * D:(h + 1) * D], in0=y2[:],
                                        in1=xsw_s[:, m * D:(m + 1) * D], op=ALU.add)
            return ob

        def st7(mb, ob):
            nc.sync.dma_start(out=out_d[:, mb * 2 * D:(mb + 1) * 2 * D], in_=ob[:])
            return None

        stages = [st2, st3, st4, st5, st6, st7]
        vals = {}

        def run_stage(s, mb):
            if not (0 <= mb < NW // QW):
                return
            if s == 0:
                vals[(1, mb)] = st2(mb, None)
            else:
                vals[(s + 1, mb)] = stages[s](mb, vals.pop((s, mb)))

        NQ = NW // QW  # 49 quads
        do_mlp = "nomlp" not in ab
        do_scatter = "noscatter" not in ab
        total_iters = NQ + len(stages) + 2
        for _rep in range(reps):
            for q in range(total_iters):
                if q < NQ and do_scatter:
                    win = winp.tile([P, QW * K * P], bf16, tag="win")
                    nc.sync.dma_start(
                        out=win[:],
                        in_=attr_d[:, q * QW * K * P:(q + 1) * QW * K * P])
                    for woff in range(QW):
                        scatter_window(q * QW + woff, win, woff)
                if do_mlp:
                    for s in range(len(stages)):
                        run_stage(s, q - 1 - s)
            vals.clear()

    nc.compile()
    return nc


def _host_shard(x, edge_index, edge_attr):
    """Route edges to destination-owning cores; build device input arrays."""
    col = np.asarray(edge_index[1]).astype(np.int64)
    E = col.shape[0]
    core = col // NPC
    local = col - core * NPC
    w = local // WIN
    r = (local % WIN).astype(np.float32)
    gw = core * NW + w
    counts = np.bincount(gw, minlength=NCORES * NW)
    K = int(np.ceil(counts.max() / P))
    K = max(K, 1)

    order = np.argsort(gw, kind="stable")
    starts = np.zeros(NCORES * NW, np.int64)
    np.cumsum(counts[:-1], out=starts[1:])
    ranks = np.arange(E, dtype=np.int64) - np.repeat(starts, counts)
    gw_s = gw[order]
    slots_per_win = K * P
    slot = gw_s * slots_per_win + ranks

    attr_s = np.asarray(edge_attr, np.float32)[order]
    hi = attr_s.astype(BF16)
    lo = (attr_s - hi.astype(np.float32)).astype(BF16)

    SLOTS = NCORES * NW * slots_per_win
    A = np.zeros((SLOTS, P), BF16)
    A[slot, 0:D] = hi
    A[slot, D:P] = lo
    R = np.zeros(SLOTS, np.float32)
    R[slot] = r[order]

    T = NW * K
    attr_T = np.ascontiguousarray(
        A.reshape(NCORES, T, P, P).transpose(0, 2, 1, 3)).reshape(NCORES, P, T * P)
    r_T = np.ascontiguousarray(
        R.reshape(NCORES, T, P).transpose(0, 2, 1)).astype(BF16)

    xpad = np.zeros((NCORES * NPC, D), np.float32)
    xpad[:N_NODES] = np.asarray(x, np.float32)
    x4 = xpad.reshape(NCORES, MT, P, D)
    xT = np.ascontiguousarray(
        x4.reshape(NCORES, NPC, D).transpose(0, 2, 1))          # [C, 64, NPC]
    x_sw = np.ascontiguousarray(
        x4.transpose(0, 2, 1, 3)).reshape(NCORES, P, MT * D)    # [C, 128, MT*64]
    return K, attr_T, r_T, xT, x_sw


def _pack_consts(W0, b0, W1, b1, W2, b2, ln_g, ln_b):
    cst = np.zeros((P, 654), np.float32)
    cst[0:D, 448:512] = np.eye(D, dtype=np.float32)
    cst[:, 512:576] = np.broadcast_to(np.asarray(ln_g, np.float32), (P, D))
    cst[:, 576:640] = np.broadcast_to(np.asarray(ln_b, np.float32), (P, D))
    cst[:, 640] = np.asarray(b0, np.float32)
    cst[:, 641] = np.asarray(b1, np.float32)
    cst[0:D, 642] = np.asarray(b2, np.float32)
    cst[:, 643] = EPS
    wr = np.zeros((P, 576), np.float32)
    W0 = np.asarray(W0, np.float32)
    wr[0:D, 0:H] = W0[0:D]                        # w0x
    wr[0:D, 128:256] = W0[D:2 * D]                # w0a_hi (zero-padded below)
    wr[D:P, 256:384] = W0[D:2 * D]                # w0a_lo (zero-padded above)
    wr[:, 384:512] = np.asarray(W1, np.float32)
    wr[:, 512:576] = np.asarray(W2, np.float32)
    return cst, wr


def kernel(x, edge_index, edge_attr, W0, b0, W1, b1, W2, b2, ln_g, ln_b):
    global last_results
    K, attr_T, r_T, xT, x_sw = _host_shard(x, edge_index, edge_attr)

    if K not in _prog_cache:
        _prog_cache[K] = _build_program(K)
    nc = _prog_cache[K]

    iota_rep = np.repeat(np.arange(WIN, dtype=np.float32), K)
    consts = {
        "iota_rep": np.ascontiguousarray(
            np.broadcast_to(iota_rep, (P, WIN * K))).astype(BF16),
    }
    consts["cst"], consts["wr"] = _pack_consts(W0, b0, W1, b1, W2, b2, ln_g, ln_b)
    in_maps = []
    for c in range(NCORES):
        m = dict(consts)
        m["attr"] = attr_T[c]
        m["r"] = r_T[c]
        m["xT"] = xT[c]
        m["x_sw"] = x_sw[c]
        in_maps.append(m)

    res = run_bass_kernel_spmd(nc, in_maps, core_ids=list(range(NCORES)))
    last_results = res
    # out_sw[p, m*64+f] -> node (m*128+p), feature f
    outs = []
    for c in range(NCORES):
        osw = res.results[c]["out_sw"].reshape(P, MT, D)
        outs.append(np.ascontiguousarray(osw.transpose(1, 0, 2)).reshape(NPC, D))
    out = np.concatenate(outs, axis=0)
    return np.ascontiguousarray(out[:N_NODES])



# revision 8
# speedup vs baseline: 3.1878x; 3.1878x over previous
"""Trainium2 Bass kernel for nn_NodeProcessor (GNN message passing).

Computation (per reference):
    agg = segment_sum(edge_attr, col=edge_index[1], N)      # [N, 64]
    h = relu(concat([x, agg]) @ W0 + b0)
    h = relu(h @ W1 + b1)
    h = h @ W2 + b2
    out = layernorm(h) * ln_g + ln_b + x

Distribution: destination-sharded edges, no collectives. Nodes are
degree-sorted globally and dealt round-robin across the 8 cores so every
core sees the IDENTICAL per-bucket degree profile (one SPMD program).
Each core owns 12800 nodes in 100 buckets of 128; bucket b is padded to
the global block-max degree d_b (zero-filled slots), giving ~1-2% pad.

Per-core device kernel:
  Scatter: edge features ship bf16 node-major [128 lanes, bucket, j, feat].
  segment-sum = (d_b-1) DVE tensor_tensor adds per bucket chunk, batched
  over all buckets of equal degree (2-byte packed operands -> DVE 2x mode).
  No one-hots, no per-128-edge matmuls.
  MLP: bf16 weights/activations, N=512 moving columns (1 cycle/row).
  agg is PE-transposed to feature-major and accumulated into layer 0 via
  a second matmul. LayerNorm: PE-transpose h2 back to node-major, grouped
  bn_stats/bn_aggr on DVE, apply as scalar-engine scale/bias, ln_g-mult
  and residual on GpSimd. ln_b is folded into the residual x on the host.
"""

import numpy as np
import ml_dtypes
from contextlib import ExitStack

from concourse import bacc, mybir
from concourse.tile import TileContext
from concourse.bass_utils import run_bass_kernel_spmd

BF16 = ml_dtypes.bfloat16

N_NODES = 100000
N_EDGES = 1200000
D = 64          # d_node == d_edge
H = 128         # hidden
EPS = 1e-5
NCORES = 8
P = 128
NPC = 12800     # nodes per core (padded)
NB = 100        # buckets of 128 nodes per core
MB = 512        # MLP mega-tile (nodes)
MT = NPC // MB  # 25 mega-tiles per core
CHCAP = 128     # chunk cap: nb*d <= CHCAP  (16KB/partition bf16)

_prog_cache: dict[tuple, object] = {}
last_results = None


def _build_program(chunks):
    """chunks: tuple of (d, b0, nb, off) with off in d-slot units."""
    F = chunks[-1][3] + chunks[-1][0] * chunks[-1][2]  # total slots per lane
    nc = bacc.Bacc("TRN2", target_bir_lowering=False, debug=False,
                   num_devices=NCORES)
    f32 = mybir.dt.float32
    bf16 = mybir.dt.bfloat16
    AF = mybir.ActivationFunctionType
    ALU = mybir.AluOpType

    attr_d = nc.dram_tensor("attr", [P, F * D], bf16, kind="ExternalInput").ap()
    xT_d = nc.dram_tensor("xT", [D, NPC], bf16, kind="ExternalInput").ap()
    xb_d = nc.dram_tensor("xb", [P, NB * D], bf16, kind="ExternalInput").ap()
    wb_d = nc.dram_tensor("wb", [P, 896], bf16, kind="ExternalInput").ap()
    cf_d = nc.dram_tensor("cf", [P, 4], f32, kind="ExternalInput").ap()
    out_d = nc.dram_tensor("out", [P, NB * D], f32, kind="ExternalOutput").ap()

    with TileContext(nc) as tc, ExitStack() as ctx:
        const = ctx.enter_context(tc.tile_pool(name="const", bufs=1))
        chp = ctx.enter_context(tc.tile_pool(name="chp", bufs=3))
        xtp = ctx.enter_context(tc.tile_pool(name="xtp", bufs=3))
        xbp = ctx.enter_context(tc.tile_pool(name="xbp", bufs=3))
        agp = ctx.enter_context(tc.tile_pool(name="agp", bufs=2))
        h0sp = ctx.enter_context(tc.tile_pool(name="h0sp", bufs=2))
        h1sp = ctx.enter_context(tc.tile_pool(name="h1sp", bufs=2))
        h2sp = ctx.enter_context(tc.tile_pool(name="h2sp", bufs=2))
        zp = ctx.enter_context(tc.tile_pool(name="zp", bufs=2))
        y1p = ctx.enter_context(tc.tile_pool(name="y1p", bufs=2))
        outp = ctx.enter_context(tc.tile_pool(name="outp", bufs=3))
        stp = ctx.enter_context(tc.tile_pool(name="stp", bufs=2))
        ps_h0 = ctx.enter_context(tc.tile_pool(name="ps_h0", bufs=2, space="PSUM"))
        ps_h1 = ctx.enter_context(tc.tile_pool(name="ps_h1", bufs=2, space="PSUM"))
        ps_h2 = ctx.enter_context(tc.tile_pool(name="ps_h2", bufs=2, space="PSUM"))
        ps_ag = ctx.enter_context(tc.tile_pool(name="ps_ag", bufs=1, space="PSUM"))
        ps_nm = ctx.enter_context(tc.tile_pool(name="ps_nm", bufs=1, space="PSUM"))

        wb = const.tile([P, 896], bf16, tag="wb")
        nc.sync.dma_start(out=wb[:], in_=wb_d[:])
        cf = const.tile([P, 4], f32, tag="cf")
        nc.sync.dma_start(out=cf[:], in_=cf_d[:])
        w0x = wb[0:D, 0:H]
        w0a = wb[0:D, H:2 * H]
        w1 = wb[:, 256:384]
        w2 = wb[:, 384:448]
        i128 = wb[:, 448:576]
        i64 = wb[0:D, 576:640]
        gt = wb[:, 640:896]          # ln_g tiled 4x, all partitions
        b0c = cf[:, 0:1]
        b1c = cf[:, 1:2]
        b2c = cf[0:D, 2:3]
        epsc = cf[:, 3:4]

        # persistent node-major aggregate, [128 lanes, bucket*64]
        acc = const.tile([P, NB * D], bf16, tag="acc")
        # persistent rotating node-major h2 (4 mega-tiles deep)
        nm = ps_nm.tile([P, 4 * 256], bf16, tag="nm")

        # ---- scatter: one chunk of nb same-degree buckets ----
        def emit_chunk(d, b0, nb, off):
            ch = chp.tile([P, nb * d * D], bf16, tag="ch", name=f"ch{b0}")
            nc.sync.dma_start(out=ch[:], in_=attr_d[:, off * D:(off + nb * d) * D])
            accv = acc[:, b0 * D:(b0 + nb) * D].rearrange(
                "p (b f) -> p b f", b=nb)
            if d == 1:
                nc.scalar.activation(out=acc[:, b0 * D:(b0 + nb) * D],
                                     in_=ch[:], func=AF.Copy)
                return
            chv = ch[:].rearrange("p (b j f) -> p b j f", b=nb, j=d)
            nc.vector.tensor_tensor(out=accv, in0=chv[:, :, 0, :],
                                    in1=chv[:, :, 1, :], op=ALU.add)
            for j in range(2, d):
                nc.vector.tensor_tensor(out=accv, in0=accv,
                                        in1=chv[:, :, j, :], op=ALU.add)

        # ---- MLP stages over 512-node mega-tiles ----
        aggTs = {}
        xts = {}
        xbs = {}
        ags = {}
        h0ss = {}
        h1ss = {}
        h2ss = {}
        stats = {}
        zs = {}
        y1s = {}

        def s0(t):
            aggT = ps_ag.tile([D, MB], bf16, tag="aggT", name=f"aggT{t}")
            for k in range(4):
                b = 4 * t + k
                nc.tensor.transpose(out=aggT[:, k * P:(k + 1) * P],
                                    in_=acc[:, b * D:(b + 1) * D],
                                    identity=i128)
            aggTs[t] = aggT
            xt = xtp.tile([D, MB], bf16, tag="xt", name=f"xt{t}")
            nc.sync.dma_start(out=xt[:], in_=xT_d[:, t * MB:(t + 1) * MB])
            xts[t] = xt

        def s1(t):
            ag = agp.tile([D, MB], bf16, tag="ag", name=f"ag{t}")
            nc.scalar.activation(out=ag[:], in_=aggTs.pop(t)[:], func=AF.Copy)
            ags[t] = ag

        def s2(t):
            h0 = ps_h0.tile([H, MB], f32, tag="h0", name=f"h0_{t}")
            nc.tensor.matmul(out=h0[:], lhsT=w0x, rhs=xts.pop(t)[:],
                             start=True, stop=False)
            nc.tensor.matmul(out=h0[:], lhsT=w0a, rhs=ags.pop(t)[:],
                             start=False, stop=True)
            return h0

        def s3(t, h0):
            h0s = h0sp.tile([H, MB], bf16, tag="h0s", name=f"h0s{t}")
            nc.scalar.activation(out=h0s[:], in_=h0[:], func=AF.Relu, bias=b0c)
            h0ss[t] = h0s

        def s4(t):
            h1 = ps_h1.tile([H, MB], f32, tag="h1", name=f"h1_{t}")
            nc.tensor.matmul(out=h1[:], lhsT=w1, rhs=h0ss.pop(t)[:],
                             start=True, stop=True)
            return h1

        def s5(t, h1):
            h1s = h1sp.tile([H, MB], bf16, tag="h1s", name=f"h1s{t}")
            nc.scalar.activation(out=h1s[:], in_=h1[:], func=AF.Relu, bias=b1c)
            h1ss[t] = h1s

        def s6(t):
            h2 = ps_h2.tile([D, MB], f32, tag="h2", name=f"h2_{t}")
            nc.tensor.matmul(out=h2[:], lhsT=w2, rhs=h1ss.pop(t)[:],
                             start=True, stop=True)
            return h2

        def s7(t, h2):
            h2s = h2sp.tile([D, MB], bf16, tag="h2s", name=f"h2s{t}")
            nc.scalar.activation(out=h2s[:], in_=h2[:], func=AF.Identity,
                                 bias=b2c)
            h2ss[t] = h2s

        def s8(t):
            o = (t % 4) * 256
            h2s = h2ss.pop(t)
            for k in range(4):
                nc.tensor.transpose(out=nm[:, o + k * D:o + (k + 1) * D],
                                    in_=h2s[:, k * P:(k + 1) * P],
                                    identity=i64)

        def s9(t):
            o = (t % 4) * 256
            st = stp.tile([P, 24 + 8 + 4 + 4 + 4 + 4], f32, tag="st",
                          name=f"st{t}")
            stv = st[:, 0:24]
            mv = st[:, 24:32]
            stdc = st[:, 32:36]
            rstd = st[:, 36:40]
            sr = st[:, 40:44]
            nb_ = st[:, 44:48]
            for g in range(4):
                nc.vector.bn_stats(out=stv[:, 6 * g:6 * g + 6],
                                   in_=nm[:, o + g * D:o + (g + 1) * D])
                nc.vector.bn_aggr(out=mv[:, 2 * g:2 * g + 2],
                                  in_=stv[:, 6 * g:6 * g + 6])
            mvv = mv.rearrange("p (g w) -> p w g", w=2)
            nc.scalar.activation(out=stdc, in_=mvv[:, 1, :], func=AF.Sqrt,
                                 bias=epsc)
            nc.vector.reciprocal(out=rstd, in_=stdc)
            nc.vector.tensor_tensor(out=sr, in0=mvv[:, 0, :], in1=rstd,
                                    op=ALU.mult)
            nc.vector.tensor_scalar_mul(out=nb_, in0=sr, scalar1=-1.0)
            stats[t] = st

        def s10(t):
            o = (t % 4) * 256
            st = stats.pop(t)
            z = zp.tile([P, 256], bf16, tag="z", name=f"z{t}")
            for g in range(4):
                nc.scalar.activation(out=z[:, g * D:(g + 1) * D],
                                     in_=nm[:, o + g * D:o + (g + 1) * D],
                                     func=AF.Identity,
                                     scale=st[:, 36 + g:37 + g],
                                     bias=st[:, 44 + g:45 + g])
            zs[t] = z
            xb = xbp.tile([P, 256], bf16, tag="xb", name=f"xb{t}")
            nc.sync.dma_start(out=xb[:], in_=xb_d[:, t * 256:(t + 1) * 256])
            xbs[t] = xb

        def s11(t):
            y1 = y1p.tile([P, 256], bf16, tag="y1", name=f"y1_{t}")
            nc.gpsimd.tensor_tensor(out=y1[:], in0=zs.pop(t)[:], in1=gt,
                                    op=ALU.mult)
            yo = outp.tile([P, 256], f32, tag="yo", name=f"yo{t}")
            nc.gpsimd.tensor_tensor(out=yo[:], in0=y1[:], in1=xbs.pop(t)[:],
                                    op=ALU.add)
            nc.sync.dma_start(out=out_d[:, t * 256:(t + 1) * 256], in_=yo[:])

        vals = {}

        def run_stage(s, t):
            if not (0 <= t < MT):
                return
            if s == 2:
                vals[(3, t)] = s2(t)
            elif s == 3:
                s3(t, vals.pop((3, t)))
            elif s == 4:
                vals[(5, t)] = s4(t)
            elif s == 5:
                s5(t, vals.pop((5, t)))
            elif s == 6:
                vals[(7, t)] = s6(t)
            elif s == 7:
                s7(t, vals.pop((7, t)))
            else:
                [s0, s1, None, None, None, None, None, None,
                 s8, s9, s10, s11][s](t)

        NS = 12
        ci = 0
        # preload chunks for the first few tiles
        while ci < len(chunks) and chunks[ci][1] < 12:
            emit_chunk(*chunks[ci])
            ci += 1
        for q in range(MT + NS):
            for s in range(NS):
                run_stage(s, q - s)
            need_b = 4 * (q + 4)
            while ci < len(chunks) and chunks[ci][1] < need_b:
                emit_chunk(*chunks[ci])
                ci += 1

    nc.compile()
    return nc


def _host_plan(col):
    """Degree-sort nodes, deal across cores, bucket, chunk."""
    NPAD = NCORES * NPC
    deg = np.zeros(NPAD, np.int64)
    deg[:N_NODES] = np.bincount(col, minlength=N_NODES)
    order = np.argsort(deg, kind="stable")          # ascending degree
    dsort = deg[order]
    d_b = dsort.reshape(NB, NCORES * P).max(axis=1)  # block max, global
    d_b = np.maximum(d_b, 1).astype(np.int64)
    pref = np.zeros(NB, np.int64)
    np.cumsum(d_b[:-1], out=pref[1:])
    # chunks: runs of equal degree, capped at CHCAP slots per lane
    chunks = []
    b = 0
    while b < NB:
        d = int(d_b[b])
        e = b
        while e < NB and d_b[e] == d:
            e += 1
        cap = max(1, CHCAP // d)
        while b < e:
            nb = min(cap, e - b)
            chunks.append((d, b, nb, int(pref[b])))
            b += nb
    return order, dsort, d_b, pref, tuple(chunks)


def _host_pack(x, col, edge_attr, order, dsort, pref, F):
    E = col.shape[0]
    NPAD = NCORES * NPC
    pos = np.empty(NPAD, np.int64)
    pos[order] = np.arange(NPAD)
    pe = pos[col]                                    # sorted-pos of each dest
    eorder = np.argsort(pe, kind="stable")
    ps = pe[eorder]
    starts = np.zeros(NPAD, np.int64)
    np.cumsum(dsort[:-1], out=starts[1:])
    j = np.arange(E, dtype=np.int64) - starts[ps]
    c = ps % NCORES
    r = ps // NCORES
    b = r // P
    lane = r % P
    rows = (c * P + lane) * F + pref[b] + j
    A = np.zeros((NCORES * P * F, D), BF16)
    A[rows] = np.asarray(edge_attr, np.float32)[eorder].astype(BF16)
    A = A.reshape(NCORES, P, F * D)
    return A


def _host_x(x, ln_b, order):
    NPAD = NCORES * NPC
    xpad = np.zeros((NPAD, D), np.float32)
    xpad[:N_NODES] = np.asarray(x, np.float32)
    idx = order[np.arange(NPC)[:, None] * NCORES + np.arange(NCORES)[None, :]]
    # idx[r, c] = node id at (core c, rank r)
    xTs, xbs = [], []
    bln = np.asarray(ln_b, np.float32)[None, :]
    for cc in range(NCORES):
        xp = xpad[idx[:, cc]]                        # [NPC, 64]
        xTs.append(np.ascontiguousarray(xp.T).astype(BF16))
        xb = (xp + bln).reshape(NB, P, D).transpose(1, 0, 2).reshape(P, NB * D)
        xbs.append(np.ascontiguousarray(xb).astype(BF16))
    return idx, xTs, xbs


def _host_consts(W0, b0, W1, b1, W2, b2, ln_g):
    wb = np.zeros((P, 896), np.float32)
    W0 = np.asarray(W0, np.float32)
    wb[0:D, 0:H] = W0[0:D]
    wb[0:D, H:2 * H] = W0[D:2 * D]
    wb[:, 256:384] = np.asarray(W1, np.float32)
    wb[:, 384:448] = np.asarray(W2, np.float32)
    wb[:, 448:576] = np.eye(P, dtype=np.float32)
    wb[0:D, 576:640] = np.eye(D, dtype=np.float32)
    wb[:, 640:896] = np.broadcast_to(
        np.tile(np.asarray(ln_g, np.float32), 4), (P, 256))
    cf = np.zeros((P, 4), np.float32)
    cf[:, 0] = np.asarray(b0, np.float32)
    cf[:, 1] = np.asarray(b1, np.float32)
    cf[0:D, 2] = np.asarray(b2, np.float32)
    cf[:, 3] = EPS
    return wb.astype(BF16), cf


def kernel(x, edge_index, edge_attr, W0, b0, W1, b1, W2, b2, ln_g, ln_b):
    global last_results
    col = np.asarray(edge_index[1]).astype(np.int64)
    order, dsort, d_b, pref, chunks = _host_plan(col)
    F = int(pref[-1] + d_b[-1])

    if chunks not in _prog_cache:
        _prog_cache[chunks] = _build_program(chunks)
    nc = _prog_cache[chunks]

    A = _host_pack(x, col, edge_attr, order, dsort, pref, F)
    idx, xTs, xbs = _host_x(x, ln_b, order)
    wb, cf = _host_consts(W0, b0, W1, b1, W2, b2, ln_g)

    in_maps = []
    for c in range(NCORES):
        in_maps.append({"attr": A[c], "xT": xTs[c], "xb": xbs[c],
                        "wb": wb, "cf": cf})

    res = run_bass_kernel_spmd(nc, in_maps, core_ids=list(range(NCORES)))
    last_results = res

    out = np.zeros((NCORES * NPC, D), np.float32)
    for c in range(NCORES):
        osw = res.results[c]["out"]                  # [128, NB*64]
        o3 = osw.reshape(P, NB, D).transpose(1, 0, 2).reshape(NPC, D)
        out[idx[:, c]] = o3
    return np.ascontiguousarray(out[:N_NODES])
